# revision 21
# baseline (speedup 1.0000x reference)
"""Causal prefill attention (B=2, H=16, L=2048, D=128, fp32 I/O) on 8 TRN2 cores.

Sharding: the 32 (b,h) pairs are split 4-per-core (data+tensor parallel on B*H);
each core runs full causal attention for its 4 heads — no collectives.

Per-head algorithm (all on one core), v6:
  - q, k, v are cast fp32->bf16 IN the load DMA (SWDGE cast, gpsimd queue —
    prefetch-only, no upstream deps) straight into SBUF; no DVE cast pass.
    v lands in the D-column slice of an augmented [128, NT, D+1] tile whose
    last column is 1.0 (softmax denominator).  NOTE: xbar DMA-transpose is
    deliberately NOT used — the tile framework serializes every DMA-transpose
    pairwise against every SWDGE DMA (HW deadlock guard), which strangles the
    staging pipeline.
  - q, k are transposed to [D, L] on the TensorEngine (identity-matmul
    transpose, 4 tile-transposes per PSUM bank); the PSUM->SBUF copies are
    split between ScalarE (KT) and VectorE (QT) to balance queue load.
  - mm1: S^T chunk = K_j (stationary [d,128]) x Q^T (moving [d, q<=512]) into
    [128, 2, 512] PSUM tiles (2 j's per batch), softmax in [k-part, q-free]
    orientation.  Raw (unscaled) scores; scale is folded into the exp.
  - exp: SPLIT ACROSS TWO ENGINES running concurrently on different batches.
    Diagonal-touching batches run on ScalarE ACTIVATE (exp, scale fused,
    exact).  Strictly-below-diagonal batches mostly run on VectorE as a
    Schraudolph fast-exp (one tensor_scalar: i16 = round(A*s + B) bit-viewed
    as bf16 ~= exp(s*scale), ~1.5% rel err), with every 3rd on ScalarE.
  - causal masking only touches diagonal 128x128 tiles: one strided
    tensor_mul per batch zeroes k>q entries of both diagonal tiles at once.
  - mm2: O_i accumulates P^T_ij x [V_j | 1] in PSUM; the ones-column
    accumulates the softmax denominator.  O tiles are packed two-per-PSUM-bank
    (merged zero-region group).
  - normalize: one reciprocal + one broadcast multiply per PSUM bank pair,
    fp32 out, stored via HWDGE DMA on the sync queue (which carries nothing
    else, so waiting on the normalize never blocks staging).
"""

import numpy as np

B, H, L, D = 2, 16, 2048, 128
NCORES = 8
HPC = (B * H) // NCORES  # heads per core = 4
NT = L // 128            # 16 k/q tiles of 128
NG = L // 512            # 4 q groups of 512
NJB = 2                  # j's batched per S psum tile / exp call
SCALE = 1.0 / float(np.sqrt(D))
# Schraudolph fast-exp: bf16_bits(exp(s*SCALE)) ~= int16(A_SCH*s + B_SCH)
A_SCH = float(SCALE * np.log2(np.e) * 128.0)
B_SCH = float(127 * 128 - 7.0)

_CACHE = {}


def _build():
    import concourse.tile as tile
    from concourse import bacc, mybir
    from concourse.bass import ts
    from concourse.masks import make_identity, make_upper_triangular

    f32 = mybir.dt.float32
    bf16 = mybir.dt.bfloat16
    i16 = mybir.dt.int16
    EXP = mybir.ActivationFunctionType.Exp

    nc = bacc.Bacc("TRN2", target_bir_lowering=False, debug=False)
    q = nc.dram_tensor("q", [HPC, L, D], f32, kind="ExternalInput").ap()
    k = nc.dram_tensor("k", [HPC, L, D], f32, kind="ExternalInput").ap()
    v = nc.dram_tensor("v", [HPC, L, D], f32, kind="ExternalInput").ap()
    out = nc.dram_tensor("out", [HPC, L, D], f32, kind="ExternalOutput").ap()

    with tile.TileContext(nc) as tc:
        with (
            tc.tile_pool(name="const", bufs=1) as cpool,
            tc.tile_pool(name="cst", bufs=2) as cstpool,
            tc.tile_pool(name="tr", bufs=2) as tpool,
            tc.tile_pool(name="vv", bufs=2) as vpool,
            tc.tile_pool(name="pt", bufs=8) as ppool,
            tc.tile_pool(name="ob", bufs=2) as opool,
            tc.tile_pool(name="stat", bufs=8) as spool,
            tc.tile_pool(name="ps_s", bufs=2, space="PSUM") as psum_s,
            tc.tile_pool(name="ps_o", bufs=1, space="PSUM") as psum_o,
            tc.tile_pool(name="ps_t", bufs=2, space="PSUM") as psum_t,
        ):
            m_ut = cpool.tile([128, 128], bf16, tag="m_ut")
            make_upper_triangular(nc, m_ut[:], val=1.0, diag=True)
            ident = cpool.tile([128, 128], bf16, tag="ident")
            make_identity(nc, ident[:])
            # preload the exp ACT table during staging (else the first real
            # exp pays the ~2.7us table load on the critical path)
            warm_act = cpool.tile([128, 1], f32, tag="warm_act")
            nc.scalar.activation(warm_act[:], m_ut[:, 0:1], EXP, scale=1.0)
            # warm the PE clock (HAM) with ~3.4us of dummy matmuls during
            # staging so the first real matmuls run at 2.4GHz (transpose-mode
            # doesn't count as PE activity, so use normal matmuls)
            warm_mm = psum_s.tile([128, NJB, 512], f32, tag="s", name="warm")
            for _ in range(32):
                nc.tensor.matmul(warm_mm[:, 0, 0:128], lhsT=ident[:],
                                 rhs=ident[:], start=True, stop=True)

            tiles = {}

            def alloc(hh):
                tiles[hh] = (
                    cstpool.tile([128, NT, D], bf16, tag="qc", name=f"qc{hh}"),
                    cstpool.tile([128, NT, D], bf16, tag="kc", name=f"kc{hh}"),
                    tpool.tile([128, L], bf16, tag="qt", name=f"qt{hh}"),
                    tpool.tile([128, L], bf16, tag="kt", name=f"kt{hh}"),
                    vpool.tile([128, NT, D + 1], bf16, tag="vb", name=f"vb{hh}"),
                )

            def t_block(src_, dstv, nm, hh, b, copy_eng):
                # transpose 4 natural 128x128 tiles into one PSUM bank, then
                # copy to the [D, L] SBUF layout
                Tp = psum_t.tile([128, 4, 128], bf16, tag="tp",
                                 name=f"tp_{nm}_{hh}_{b}")
                for u in range(4):
                    nc.tensor.matmul(
                        Tp[:, u, :], lhsT=src_[:, 4 * b + u, :],
                        rhs=ident[:], is_transpose=True,
                        start=(u == 0), stop=(u == 3),
                    )
                copy_eng(dstv[:, 512 * b : 512 * (b + 1)], Tp[:])

            def stage_q(hh, b, full=False):
                # cast-DMA k/q/v slices in (one 512-row quarter, or the whole
                # head when full=True), then PE-transpose k/q into KT/QT
                Qc, Kc, QT, KT, Vb = tiles[hh]
                tsl = slice(0, NT) if full else slice(4 * b, 4 * b + 4)
                kv = k[hh].rearrange("(t p) d -> p t d", p=128)
                qv = q[hh].rearrange("(t p) d -> p t d", p=128)
                vt = v[hh].rearrange("(t p) d -> p t d", p=128)
                if b == 0:
                    nc.vector.memset(Vb[:, :, D : D + 1], 1.0)
                nc.gpsimd.dma_start(Qc[:, tsl, :], qv[:, tsl, :])
                nc.gpsimd.dma_start(Kc[:, tsl, :], kv[:, tsl, :])
                nc.gpsimd.dma_start(Vb[:, tsl, 0:D], vt[:, tsl, :])
                for bb in ([0, 1, 2, 3] if full else [b]):
                    t_block(Kc, KT, "k", hh, bb, nc.scalar.copy)
                    t_block(Qc, QT, "q", hh, bb, nc.vector.tensor_copy)

            alloc(0)
            for b4 in range(NG):
                stage_q(0, b4)

            for hh in range(HPC):
                Qc, Kc, QT, KT, Vb = tiles[hh]
                for g in range(NG):
                    nsched = 0  # below-diag batch counter (S,V,S,V... order)
                    if g == 1 and hh + 1 < HPC:
                        # emit next head's staging early so its DMAs and
                        # transposes run during this head's remaining compute
                        alloc(hh + 1)
                        for b4 in range(NG):
                            stage_q(hh + 1, b4)
                    nj = 4 * g + 4  # k tiles for this q group
                    # 4 O accumulators packed 2-per-bank: Opk[u][:, r2, :]
                    Opk = [
                        psum_o.tile([128, 2, D + 1], f32, tag=f"opk{u}",
                                    name=f"opk{u}_{hh}_{g}")
                        for u in range(2)
                    ]

                    for jb0 in range(0, nj, NJB):
                        jbn = min(NJB, nj - jb0)  # j's in this batch
                        S = psum_s.tile([128, NJB, 512], f32, tag="s")
                        PT = ppool.tile([128, NJB, 512], bf16, tag="pt")
                        # chunk start for the whole batch: union of live
                        # columns (so the batched exp never reads unwritten
                        # PSUM; sub-diagonal surplus is computed and ignored)
                        c0 = 128 * max(0, jb0 - 4 * g)
                        for jj in range(jbn):
                            j = jb0 + jj
                            nc.tensor.matmul(
                                S[:, jj, c0:512],
                                lhsT=KT[:, ts(j, 128)],
                                rhs=QT[:, g * 512 + c0 : (g + 1) * 512],
                                start=True,
                                stop=True,
                            )
                        below_diag = jb0 + jbn - 1 < 4 * g
                        # strict V/S alternation so consecutive batches run
                        # their exps on different engines concurrently:
                        # below-diag alternates S,V,S,V,... then diag1 (the
                        # exact biggest-weight tiles) on ScalarE and diag2 on
                        # VectorE (Schraudolph, validated rel-err ~8e-3)
                        if below_diag:
                            use_v = nsched % 2 == 1
                            nsched += 1
                        else:
                            use_v = jb0 == 4 * g + 2
                        if use_v:
                            # Schraudolph fast exp on the vector engine (raw
                            # scores in, bf16 bit-pattern out via int16);
                            # one call per j so mm2 can start on j0's P
                            # while j1's exp still runs
                            for jj in range(jbn):
                                nc.vector.tensor_scalar(
                                    PT[:, jj, c0:512].bitcast(i16),
                                    S[:, jj, c0:512],
                                    A_SCH, B_SCH,
                                    mybir.AluOpType.mult,
                                    mybir.AluOpType.add,
                                )
                        else:
                            # one call per j so mm2 can start on j0's P while
                            # j1's exp still runs
                            for jj in range(jbn):
                                nc.scalar.activation(
                                    PT[:, jj, c0:512], S[:, jj, c0:512],
                                    EXP, scale=SCALE,
                                )
                        if not below_diag:
                            # zero k>q of both diagonal tiles (jj, r0=
                            # jj+(jb0-4g)) in ONE strided op: in 128-col
                            # blocks of the flat PT they sit at blocks
                            # {b0, b0+5} where b0 = jb0-4g
                            b0 = jb0 - 4 * g  # 0 or 2
                            blk = PT[:].rearrange(
                                "p a (c d) -> p (a c) d", d=128
                            )
                            mt = blk[:, b0 : b0 + 6 : 5, :]
                            nc.vector.tensor_mul(
                                mt, mt,
                                m_ut[:, None, :].broadcast_to(
                                    [128, 2, 128]),
                            )
                        for jj in range(jbn):
                            j = jb0 + jj
                            r0 = max(0, j - 4 * g)
                            for r in range(r0, 4):
                                i = 4 * g + r
                                # two O accumulators share each PSUM bank; the
                                # bank's zero-region group is started by the
                                # first matmul (r even, j=0 zeroes the whole
                                # bank) and stopped by the last (r odd, j=i)
                                nc.tensor.matmul(
                                    Opk[r // 2][:, r % 2, :],
                                    lhsT=PT[:, jj, ts(r, 128)],
                                    rhs=Vb[:, j, :],
                                    start=(j == 0 and r % 2 == 0),
                                    stop=(j == i and r % 2 == 1),
                                )

                    Og = opool.tile([128, 4, D], f32, tag="og")
                    for u in range(2):
                        linv = spool.tile([128, 2], f32, tag="linv")
                        nc.vector.reciprocal(linv[:], Opk[u][:, :, D])
                        nc.vector.tensor_mul(
                            Og[:, 2 * u : 2 * u + 2, :],
                            Opk[u][:, :, 0:D],
                            linv[:, :, None].broadcast_to([128, 2, D]),
                        )
                    nc.sync.dma_start(
                        out[hh, g * 512 : (g + 1) * 512, :].rearrange(
                            "(r p) d -> p r d", p=128
                        ),
                        Og[:],
                    )

    nc.compile()
    return nc


def _get_nc():
    if "nc" not in _CACHE:
        _CACHE["nc"] = _build()
    return _CACHE["nc"]


def kernel(q, k, v):
    from concourse.bass_utils import run_bass_kernel_spmd

    nc = _get_nc()

    qf = np.ascontiguousarray(q, dtype=np.float32).reshape(B * H, L, D)
    kf = np.ascontiguousarray(k, dtype=np.float32).reshape(B * H, L, D)
    vf = np.ascontiguousarray(v, dtype=np.float32).reshape(B * H, L, D)

    in_maps = [
        {
            "q": qf[c * HPC : (c + 1) * HPC],
            "k": kf[c * HPC : (c + 1) * HPC],
            "v": vf[c * HPC : (c + 1) * HPC],
        }
        for c in range(NCORES)
    ]
    try:
        res = run_bass_kernel_spmd(nc, in_maps, core_ids=list(range(NCORES)))
    except Exception:
        # transient NRT/device hiccups are usually cleared by a retry
        res = run_bass_kernel_spmd(nc, in_maps, core_ids=list(range(NCORES)))
    full = np.concatenate(
        [np.asarray(res.results[c]["out"]) for c in range(NCORES)], axis=0
    )
    return full.reshape(B, H, L, D).astype(np.float32)
